# revision 27
# baseline (speedup 1.0000x reference)
"""Multi-head attention with "restricted softmax" on 8 TRN2 NeuronCores.

Reference computation (per head):
    score = Q @ K.T / sqrt(D)                       # [S, S]
    attn  = exp(score) / (1 + sum_k exp(score))     # restricted softmax
    out   = attn @ V                                # [S, D]

Full problem: B=2, H=16, S=2048, D=64  ->  32 heads, 4 heads per core.

Per-core kernel strategy (no communication needed):
  - Scores computed TRANSPOSED (S^T[k, q]) so softmax's k-reduction sits on
    the PSUM partition axis where the PE performs it for free: PV uses
    lhsT=[V | 1] so the extra output row is sum_k exp = the denominator.
  - The scores contraction is only d=64, so TWO k-tiles run CONCURRENTLY as
    row-tiled K=64 matmuls (tile_position (0,0) / (64,0)): qT/kT rows 64-127
    hold a DUPLICATE of rows 0-63 (instead of zero padding) so the second
    row-group has data to read.  Measured 106 ns per N=512 matmul in pairs
    vs 216 solo -- the scores phase runs at 2 cols/cycle.
  - Passes are QH=512 q-columns wide; a score pair lands in ONE [128, 1024]
    PSUM tile (2 banks), so the ScalarEngine exps a whole pair per ACTIVATE
    (halves the 352-cycle per-call overhead).  PSUM: 3 pair slots (6 banks)
    + double-buffered oT [65, 512] (2 banks) = 8.
  - exp split per pass: 5 pairs on ScalarE, 3 pairs on the VectorEngine via
    a Schraudolph-style fp16 bit-pattern exp (i16 = s*(log2e*1024/8) +
    (15*1024 - 59.3), bitcast to fp16 == exp(score/8), +-2% mantissa ripple).
  - ~10 dummy matmuls on garbage data at kernel start warm the PE's HAM
    clock gate during the DMA ramp, so the real matmul stream never runs
    at the cold 1.2 GHz clock.
  - Q/K transposes for heads 1-3 ride the DMA X-bar via a DRAM bounce
    (fp16 [S, 128] with both column halves = the data).  Head 0 uses
    TensorEngine transposes of duplicated-half [128, 128] tiles, emitted
    interleaved with pass-0/1 iterations so they never serialize ahead of
    the scores stream.
  - Epilogue per pass: evict oT to fp16, X-bar transpose back to [q, d],
    normalize (reciprocal on DVE, the per-block multiplies on the
    otherwise-idle GpSimd engine), DMA out.  The final pass's epilogue is
    split in halves so its stages pipeline instead of chaining serially.
"""

import math
import os

import numpy as np

import concourse.bass as bass  # noqa: F401  (bass must import before tile)
import concourse.mybir as mybir
import concourse.tile as tile
from concourse import bacc
from concourse.bass_utils import run_bass_kernel_spmd
from concourse.masks import make_identity

B, H, S, D = 2, 16, 2048, 64
N_CORES = 8
HPC = (B * H) // N_CORES  # heads per core = 4

F32 = mybir.dt.float32
F16 = mybir.dt.float16
I16 = mybir.dt.int16
EXP = mybir.ActivationFunctionType.Exp

SCALE = 1.0 / 8.0  # 1/sqrt(D)
NQ = S // 128      # 16 tiles of 128 along both q and k
QH = 512           # q-columns per pass
NP = NQ // 2       # 8 score pairs per pass

# score pairs (of 8 per pass) whose exp runs on the VectorEngine bit-trick
OFF_PAIRS = (1, 4, 6)
EXP_A = math.log2(math.e) * 1024.0 / 8.0          # 184.665 (includes 1/sqrt(D))
EXP_B = 15.0 * 1024.0 - 59.29                     # mean-centering constant

# head-0 ramp transposes emitted inside pass iterations: (pass_idx, iter) ->
# list of ("q"|"k", tile).  Initial burst before pass 0: k0-5, q0-3.
RAMP_TP = {
    (0, 0): [("k", 6), ("k", 7)],
    (0, 1): [("k", 8), ("k", 9)],
    (0, 2): [("k", 10), ("k", 11)],
    (0, 3): [("k", 12), ("k", 13), ("q", 4)],
    (0, 4): [("k", 14), ("k", 15), ("q", 5)],
    (0, 5): [("q", 6)],
    (0, 6): [("q", 7)],
    (1, 0): [("q", 8), ("q", 9)],
    (1, 1): [("q", 10), ("q", 11)],
    (1, 2): [("q", 12), ("q", 13)],
    (1, 3): [("q", 14), ("q", 15)],
}


class _HeadInputs:
    """Per-head staged inputs: fp16 Q^T/K^T [128, S] (rows 0..63 data, rows
    64..127 a DUPLICATE of the data so row-tiled pair matmuls can read both
    halves) and [V | 1].

    Heads 1-3: transposes run on the DMA X-bar through a DRAM bounce: fp32
    load -> fp16 cast -> two half-column DRAM stores (data + duplicate) ->
    one transposed load.  Zero PE cost, same DMA bytes as a zero-padded
    bounce."""

    def __init__(self, ctx, h):
        self.ctx = ctx
        self.h = h

    def emit_transpose(self, kind, n):
        """Head-0 ramp transpose on the TensorEngine.  q16/k16 tiles carry
        the duplicate in columns 64-127, so one full [128, 128] transpose
        lands data + duplicate rows in one shot.  PSUM is fully booked, so
        the output borrows an f16 bitcast view of a scores pair slot."""
        nc, pools = self.ctx["nc"], self.ctx
        st16, tT = (self.q16, self.qT) if kind == "q" else (self.k16, self.kT)
        host = pools["ps_s_pool"].tile([128, QH * 2], F32, tag="s", name=f"tp_{kind}{n}")
        tp = host[:, :64].bitcast(F16)
        nc.tensor.transpose(tp, st16[:, n, :], pools["ident16"][:])
        nc.vector.tensor_copy(tT[:, n * 128:(n + 1) * 128], tp)

    def start_dma_split(self):
        """Head-0 ramp: PE is idle, so transpose on the TensorEngine
        (shorter critical chain than the DRAM bounce).  Casts write BOTH
        column halves (data + duplicate) so the transposes are full-tile."""
        nc, pools, h = self.ctx["nc"], self.ctx, self.h
        head_pool = pools["head_pool"]
        qkt_pool = pools["qkt_pool"]

        q_nat = head_pool.tile([128, NQ, D], F32, tag="q_nat", name=f"q_nat{h}")
        k_nat = head_pool.tile([128, NQ, D], F32, tag="k_nat", name=f"k_nat{h}")
        v_nat = head_pool.tile([128, NQ, D], F32, tag="v_nat", name=f"v_nat{h}")
        q16 = head_pool.tile([128, NQ, 128], F16, tag="q16", name=f"q16_{h}")
        k16 = head_pool.tile([128, NQ, 128], F16, tag="k16", name=f"k16_{h}")
        self.q16, self.k16 = q16, k16
        self.qT = qkt_pool.tile([128, S], F16, tag="qT", name=f"qT{h}")
        self.kT = qkt_pool.tile([128, S], F16, tag="kT", name=f"kT{h}")
        qd = pools["q_dram"][h].rearrange("(n p) d -> p n d", p=128)
        kd = pools["k_dram"][h].rearrange("(n p) d -> p n d", p=128)
        chunks = ((k_nat, k16, kd, 0, 6), (q_nat, q16, qd, 0, 4),
                  (k_nat, k16, kd, 6, NQ), (q_nat, q16, qd, 4, NQ))
        for ci, (nat, st16, dr, a, b) in enumerate(chunks):
            nc.sync.dma_start(nat[:, a:b, :], dr[:, a:b, :])
            nc.vector.tensor_copy(st16[:, a:b, :D], nat[:, a:b, :])
            nc.vector.tensor_copy(st16[:, a:b, D:], nat[:, a:b, :])
            if ci == 1:
                # V rides after the first critical chunks; PV k=0 needs it
                # only once exp of the first pair lands
                nc.sync.dma_start(
                    v_nat[:],
                    pools["v_dram"][h].rearrange("(n p) d -> p n d", p=128),
                )
        # transposes that unblock the first few score pairs; the rest are
        # emitted inside pass-0/1 iterations (RAMP_TP)
        for kind, n in (("k", 0), ("k", 1), ("q", 0), ("q", 1), ("q", 2),
                        ("q", 3), ("k", 2), ("k", 3), ("k", 4), ("k", 5)):
            self.emit_transpose(kind, n)
        v1 = head_pool.tile([128, NQ, D + 1], F16, tag="v1", name=f"v1_{h}")
        nc.vector.tensor_copy(
            v1[:, :, D:].rearrange("p n one -> p (n one)"),
            pools["ones"][:],
        )
        nc.vector.tensor_copy(v1[:, :, :D], v_nat[:])
        self.v1 = v1

    def stage(self, i):
        """Emit staging stage i (0..5).  The stages are spread across passes
        so every SP DMA trigger's producer (the slow GpSimd casts above all)
        has completed by the time the in-order SP queue reaches it:
          0: q load + q casts   1: k load + k casts   2: v load + q store
          3: v1 build           4: k store + q X-bar  5: k X-bar
        Casts write BOTH column halves (data + duplicate) on the
        otherwise-idle GpSimd engine, a full head ahead of use."""
        nc, pools, h = self.ctx["nc"], self.ctx, self.h
        head_pool = pools["head_pool"]
        dram_pool = pools["dram_pool"]
        qkt_pool = pools["qkt_pool"]

        # head 1 is ramp-critical: its lo-half casts + v1 run on the (then
        # mostly idle) ScalarEngine concurrently with GpSimd's hi-half casts
        fast = nc.scalar.copy if self.h == 1 else nc.gpsimd.tensor_copy
        if i == 0:
            self.q_nat = head_pool.tile([128, NQ, D], F32, tag="q_nat", name=f"q_nat{h}")
            self.k_nat = head_pool.tile([128, NQ, D], F32, tag="k_nat", name=f"k_nat{h}")
            self.v_nat = head_pool.tile([128, NQ, D], F32, tag="v_nat", name=f"v_nat{h}")
            self.q16 = head_pool.tile([128, NQ, 128], F16, tag="q16", name=f"q16_{h}")
            self.k16 = head_pool.tile([128, NQ, 128], F16, tag="k16", name=f"k16_{h}")
            self.qdr = dram_pool.tile([S, 128], F16, tag="qdr", name=f"qdr{h}")
            self.kdr = dram_pool.tile([S, 128], F16, tag="kdr", name=f"kdr{h}")
            self.qT = qkt_pool.tile([128, S], F16, tag="qT", name=f"qT{h}")
            self.kT = qkt_pool.tile([128, S], F16, tag="kT", name=f"kT{h}")
            nc.sync.dma_start(
                self.q_nat[:],
                pools["q_dram"][h].rearrange("(n p) d -> p n d", p=128),
            )
            fast(self.q16[:, :, :D], self.q_nat[:])
            nc.gpsimd.tensor_copy(self.q16[:, :, D:], self.q_nat[:])
        elif i == 1:
            nc.sync.dma_start(
                self.k_nat[:],
                pools["k_dram"][h].rearrange("(n p) d -> p n d", p=128),
            )
            fast(self.k16[:, :, :D], self.k_nat[:])
            nc.gpsimd.tensor_copy(self.k16[:, :, D:], self.k_nat[:])
        elif i == 2:
            nc.sync.dma_start(
                self.v_nat[:],
                pools["v_dram"][h].rearrange("(n p) d -> p n d", p=128),
            )
            nc.sync.dma_start(
                self.qdr[:].rearrange("(n p) c -> p n c", p=128), self.q16[:]
            )
        elif i == 3:
            v1 = head_pool.tile([128, NQ, D + 1], F16, tag="v1", name=f"v1_{h}")
            fast(
                v1[:, :, D:].rearrange("p n one -> p (n one)"), pools["ones"][:]
            )
            fast(v1[:, :, :D], self.v_nat[:])
            self.v1 = v1
        elif i == 4:
            nc.sync.dma_start(
                self.kdr[:].rearrange("(n p) c -> p n c", p=128), self.k16[:]
            )
            nc.sync.dma_start_transpose(self.qT[:], self.qdr[:])
        elif i == 5:
            nc.sync.dma_start_transpose(self.kT[:], self.kdr[:])


def _attention(tc):
    nc = tc.nc
    q_dram = nc.dram_tensor("query", [HPC, S, D], F32, kind="ExternalInput").ap()
    k_dram = nc.dram_tensor("key", [HPC, S, D], F32, kind="ExternalInput").ap()
    v_dram = nc.dram_tensor("value", [HPC, S, D], F32, kind="ExternalInput").ap()
    o_dram = nc.dram_tensor("out", [HPC, S, D], F32, kind="ExternalOutput").ap()

    with (
        tc.tile_pool(name="const", bufs=1) as const_pool,
        tc.tile_pool(name="head_io", bufs=3) as head_pool,
        tc.tile_pool(name="qkt", bufs=3) as qkt_pool,
        tc.tile_pool(name="et", bufs=4) as et_pool,
        tc.tile_pool(name="eti", bufs=2) as eti_pool,
        tc.tile_pool(name="epi", bufs=2) as epi_pool,
        tc.tile_pool(name="dram", bufs=2, space="DRAM") as dram_pool,
        tc.tile_pool(name="ps_s", bufs=3, space="PSUM") as ps_s_pool,
        tc.tile_pool(name="ps_o", bufs=2, space="PSUM") as ps_o_pool,
    ):
        ident16 = const_pool.tile([128, 128], F16)
        make_identity(nc, ident16[:])
        ones = const_pool.tile([128, NQ], F16)
        nc.vector.memset(ones[:], 1.0)
        wsrc = const_pool.tile([128, 512], F16)
        nc.vector.memset(wsrc[:], 0.015625)
        # [1, 65] unit row (1 at col 64) and [1, 512] ones: a K=1 matmul of
        # these inside the PV accumulation adds the restricted softmax's +1
        # to the denominator row of oT for free
        e65 = const_pool.tile([1, D + 1], F16)
        nc.vector.memset(e65[:], 0.0)
        nc.vector.memset(e65[:, D:], 1.0)
        ones512 = const_pool.tile([1, QH], F16)
        nc.vector.memset(ones512[:], 1.0)

        # ~10 dummy matmuls warm the PE's HAM clock gate (~3.4us of PE busy
        # flips it to 2.4 GHz) while the first DMAs land
        warm_ps = ps_s_pool.tile([128, QH * 2], F32, tag="s", name="warm")
        for i in range(10):
            nc.tensor.matmul(
                warm_ps[:, :512], wsrc[:, :128], wsrc[:],
                start=True, stop=True,
            )

        ctx = {
            "nc": nc, "q_dram": q_dram, "k_dram": k_dram, "v_dram": v_dram,
            "head_pool": head_pool, "qkt_pool": qkt_pool,
            "ps_s_pool": ps_s_pool, "dram_pool": dram_pool,
            "ident16": ident16, "ones": ones,
        }

        heads = [_HeadInputs(ctx, h) for h in range(HPC)]
        heads[0].start_dma_split()

        def emit_pair(hd, qh, p, sp):
            """Two k-tiles (2p, 2p+1) of scores, concurrently as K=64
            row-tiles at tile_position (0,0) / (64,0)."""
            q0 = qh * QH
            j = 2 * p
            nc.tensor.matmul(
                sp[:, 0:QH],
                hd.kT[0:64, j * 128:(j + 1) * 128],
                hd.qT[0:64, q0:q0 + QH],
                start=True, stop=True,
            )
            nc.tensor.matmul(
                sp[:, QH:2 * QH],
                hd.kT[64:128, (j + 1) * 128:(j + 2) * 128],
                hd.qT[64:128, q0:q0 + QH],
                start=True, stop=True,
            )

        def emit_pv(hd, oT, p, et_ap):
            for j in (0, 1):
                k = 2 * p + j
                nc.tensor.matmul(
                    oT[:D + 1, :],
                    hd.v1[:, k, :],
                    et_ap[:, j * QH:(j + 1) * QH],
                    start=(k == 0), stop=(k == NQ - 1),
                )
                if k == 0:
                    # denominator += 1 (rank-1: e65.T @ ones512 hits row 64)
                    nc.tensor.matmul(
                        oT[:D + 1, :], e65[:], ones512[:],
                        start=False, stop=False,
                    )

        def epi_copy(st, half=None):
            """Stage A: evict oT PSUM -> fp16 SBUF (DVE), and drop the fp16
            reciprocal of the denominator into row 65 so the X-bar transpose
            delivers it per-q.  Depends only on PSUM -- never on a DMA."""
            if "oT16" not in st:
                st["oT16"] = epi_pool.tile([80, QH], F16, tag="oT16", name="oT16")
            hs = slice(None) if half is None else slice(half * (QH // 2), (half + 1) * (QH // 2))
            nc.vector.tensor_copy(st["oT16"][:D + 1, hs], st["oT"][:D + 1, hs])
            # in-place fp16 reciprocal of the denominator row (partition 64,
            # 32-aligned; engine APs cannot start at unaligned partitions)
            with nc.allow_low_precision(reason="fp16 recip: 5e-4 rel, gate is 2e-2"):
                nc.vector.reciprocal(st["oT16"][D:D + 1, hs], st["oT16"][D:D + 1, hs])

        def epi_transpose(st, half=None):
            """Stage B: X-bar transpose [80, cols] -> [128, cols//128, 80]."""
            if "trT" not in st:
                st["trT"] = epi_pool.tile([128, QH // 128, 80], F16, tag="trT", name="trT")
            nb = QH // 128
            bs = slice(None) if half is None else slice(half * (nb // 2), (half + 1) * (nb // 2))
            hs = slice(None) if half is None else slice(half * (QH // 2), (half + 1) * (QH // 2))
            nc.sync.dma_start_transpose(st["trT"][:, bs, :], st["oT16"][:, hs])

        def epi_normalize(st, half=None):
            """Stage C: one GpSimd multiply by the transposed reciprocal
            (trT row 65) + out DMA.  On GpSimd because it waits on the trT
            transpose DMA: a DMA-waiting op on the DVE queue would block the
            etis behind it and starve the PE's PV matmuls; on GpSimd the
            only things queued behind are slack-rich casts."""
            h, qh, trT = st["h"], st["qh"], st["trT"]
            nb = QH // 128
            bs = range(nb) if half is None else range(half * nb // 2, (half + 1) * nb // 2)
            if "o_sb" not in st:
                st["o_sb"] = epi_pool.tile([128, nb, D], F32, tag="o_sb", name="o_sb")
            o_sb = st["o_sb"]
            j0, j1 = min(bs), max(bs) + 1
            nc.gpsimd.tensor_tensor(
                o_sb[:, j0:j1, :], trT[:, j0:j1, :D],
                trT[:, j0:j1, D].broadcast_to([128, j1 - j0, D]),
                mybir.AluOpType.mult,
            )
            nc.sync.dma_start(
                o_dram[h].rearrange("(n p) d -> p n d", p=128)[:, qh * nb + j0:qh * nb + j1, :],
                o_sb[:, j0:j1, :],
            )

        # staging stage schedule: (pass_idx -> [(head, stage)]).  Spaced so
        # every SP trigger's producer (slow GpSimd casts, DMA transfers) has
        # completed by the time the in-order SP queue reaches it, and each
        # head's X-bar transposes land >=1 pass before its first use.
        stage_at = {}
        for hh in range(1, HPC):
            pl = {1: [0, 0, 1, 1, 2, 2],
                  2: [1, 2, 3, 4, 4, 5],
                  3: [4, 5, 6, 7, 8, 9]}[hh]
            for si, pp in enumerate(pl):
                stage_at.setdefault(pp, []).append((hh, si))

        pending_epi = []
        passes = [(h, qh) for h in range(HPC) for qh in range(S // QH)]

        def new_pair(idx2, p2):
            """Allocate + emit score pair p2 of pass idx2."""
            h2, qh2 = passes[idx2]
            sp = ps_s_pool.tile([128, QH * 2], F32, tag="s", name=f"sp{idx2}_{p2}")
            emit_pair(heads[h2], qh2, p2, sp)
            return sp

        # score pairs are produced TWO iterations ahead of their exp: the
        # PE's in-order queue stalls on PV-waiting-for-exp, so a pair only
        # one iteration ahead would be trapped behind that stall and the
        # exp engines would idle.  3 PSUM pair slots = exactly pipeline
        # depth 3 (being exp'd / waiting / being produced).
        carry = [new_pair(0, 0), new_pair(0, 1)]
        for idx, (h, qh) in enumerate(passes):
            hd = heads[h]
            for hh, si in stage_at.get(idx, ()):
                heads[hh].stage(si)

            oT = ps_o_pool.tile([D + 1, QH], F32, tag="oT", name="oT")
            pair_tiles = {0: carry[0], 1: carry[1]}
            carry = []

            for p in range(NP):
                for kind, n in RAMP_TP.get((idx, p), ()):
                    heads[0].emit_transpose(kind, n)
                sp = pair_tiles.pop(p)
                if p in OFF_PAIRS:
                    eti = eti_pool.tile([128, QH * 2], I16, tag="eti", name=f"eti{p}")
                    nc.vector.tensor_scalar(
                        eti[:], sp[:], EXP_A, EXP_B,
                        mybir.AluOpType.mult, mybir.AluOpType.add,
                    )
                    et_ap = eti[:].bitcast(F16)
                else:
                    et = et_pool.tile([128, QH * 2], F16, tag="et", name=f"et{p}")
                    nc.scalar.activation(et[:], sp[:], EXP, scale=SCALE)
                    et_ap = et[:]
                if p + 2 < NP:
                    pair_tiles[p + 2] = new_pair(idx, p + 2)
                elif idx + 1 < len(passes):
                    # hoist the next pass's first pairs into this pass's
                    # tail so the exp engines never idle at the boundary
                    carry.append(new_pair(idx + 1, p + 2 - NP))
                # drain the previous pass's epilogue in stages so each DMA
                # trigger's dependency is met when the in-order SP queue
                # reaches it
                if pending_epi:
                    if p == 1:
                        epi_copy(pending_epi[0])
                    elif p == 2:
                        epi_transpose(pending_epi[0])
                    elif p == 7:
                        epi_normalize(pending_epi.pop(0))
                emit_pv(hd, oT, p, et_ap)
            pending_epi.append({"h": h, "qh": qh, "oT": oT})
        # tail: pipeline the final epilogue(s) in q-halves so the DVE copy,
        # X-bar transpose and normalize overlap instead of chaining serially
        for st in pending_epi:
            epi_copy(st, half=0)
            epi_transpose(st, half=0)
            epi_copy(st, half=1)
            epi_normalize(st, half=0)
            epi_transpose(st, half=1)
            epi_normalize(st, half=1)


_NC_CACHE = None
_TRACE_READY = False


def _enable_tracing():
    """Register the NTFF profile hook that this image's antenv lacks, and
    keep profiling artifacts local instead of uploading to a bucket."""
    global _TRACE_READY
    if _TRACE_READY:
        return
    import sys
    import types

    import antenv
    import concourse.bass_utils as bu
    from trn_agent_boot.trn_boot import _ntff_profile_via_ctypes

    if "antenv.axon_hooks" not in sys.modules:
        mod = types.ModuleType("antenv.axon_hooks")
        mod._hook = None

        def set_axon_ntff_profile_hook(h):
            mod._hook = h

        def get_axon_ntff_profile_hook():
            return mod._hook

        mod.set_axon_ntff_profile_hook = set_axon_ntff_profile_hook
        mod.get_axon_ntff_profile_hook = get_axon_ntff_profile_hook
        sys.modules["antenv.axon_hooks"] = mod
        antenv.axon_hooks = mod

    hooks = sys.modules["antenv.axon_hooks"]
    if hooks.get_axon_ntff_profile_hook() is None:
        hooks.set_axon_ntff_profile_hook(
            _ntff_profile_via_ctypes("/opt/axon/libaxon_pjrt.so")
        )
    bu.upload_artifacts = lambda tmpdir: tmpdir
    _TRACE_READY = True


def _build():
    global _NC_CACHE
    if _NC_CACHE is None:
        nc = bacc.Bacc("TRN2", target_bir_lowering=False, debug=False)
        with tile.TileContext(nc) as tc:
            _attention(tc)
        nc.compile()
        _NC_CACHE = nc
    return _NC_CACHE


def _run(query, key, value, trace=False, tmpdir=None):
    if trace:
        _enable_tracing()
    q = np.ascontiguousarray(np.asarray(query, dtype=np.float32).reshape(B * H, S, D))
    k = np.ascontiguousarray(np.asarray(key, dtype=np.float32).reshape(B * H, S, D))
    v = np.ascontiguousarray(np.asarray(value, dtype=np.float32).reshape(B * H, S, D))
    in_maps = [
        {
            "query": q[c * HPC:(c + 1) * HPC],
            "key": k[c * HPC:(c + 1) * HPC],
            "value": v[c * HPC:(c + 1) * HPC],
        }
        for c in range(N_CORES)
    ]
    nc = _build()
    res = run_bass_kernel_spmd(
        nc, in_maps, core_ids=list(range(N_CORES)), trace=trace, tmpdir=tmpdir
    )
    out = np.stack([res.results[c]["out"] for c in range(N_CORES)])  # [8, HPC, S, D]
    return out.reshape(B, H, S, D), res


def kernel(query, key, value):
    out, _ = _run(query, key, value, trace=bool(int(os.environ.get("BASS_TRACE", "0"))))
    return out


# revision 28
# speedup vs baseline: 1.0697x; 1.0697x over previous
"""Multi-head attention with "restricted softmax" on 8 TRN2 NeuronCores.

Reference computation (per head):
    score = Q @ K.T / sqrt(D)                       # [S, S]
    attn  = exp(score) / (1 + sum_k exp(score))     # restricted softmax
    out   = attn @ V                                # [S, D]

Full problem: B=2, H=16, S=2048, D=64  ->  32 heads, 4 heads per core.

Per-core kernel strategy (no communication needed):
  - Scores computed TRANSPOSED (S^T[k, q]) so softmax's k-reduction sits on
    the PSUM partition axis where the PE performs it for free: PV uses
    lhsT=[V | 1] so the extra output row is sum_k exp = the denominator.
  - The scores contraction is only d=64, so TWO k-tiles run CONCURRENTLY as
    row-tiled K=64 matmuls (tile_position (0,0) / (64,0)): qT/kT rows 64-127
    hold a DUPLICATE of rows 0-63 (instead of zero padding) so the second
    row-group has data to read.  Measured 106 ns per N=512 matmul in pairs
    vs 216 solo -- the scores phase runs at 2 cols/cycle.
  - Passes are QH=512 q-columns wide; a score pair lands in ONE [128, 1024]
    PSUM tile (2 banks), so the ScalarEngine exps a whole pair per ACTIVATE
    (halves the 352-cycle per-call overhead).  PSUM: 3 pair slots (6 banks)
    + double-buffered oT [65, 512] (2 banks) = 8.
  - exp split per pass: 5 pairs on ScalarE, 3 pairs on the VectorEngine via
    a Schraudolph-style fp16 bit-pattern exp (i16 = s*(log2e*1024/8) +
    (15*1024 - 59.3), bitcast to fp16 == exp(score/8), +-2% mantissa ripple).
  - ~10 dummy matmuls on garbage data at kernel start warm the PE's HAM
    clock gate during the DMA ramp, so the real matmul stream never runs
    at the cold 1.2 GHz clock.
  - Q/K transposes for heads 1-3 ride the DMA X-bar via a DRAM bounce
    (fp16 [S, 128] with both column halves = the data).  Head 0 uses
    TensorEngine transposes of duplicated-half [128, 128] tiles, emitted
    interleaved with pass-0/1 iterations so they never serialize ahead of
    the scores stream.
  - Epilogue per pass: evict oT to fp16, X-bar transpose back to [q, d],
    normalize (reciprocal on DVE, the per-block multiplies on the
    otherwise-idle GpSimd engine), DMA out.  The final pass's epilogue is
    split in halves so its stages pipeline instead of chaining serially.
"""

import math
import os

import numpy as np

import concourse.bass as bass  # noqa: F401  (bass must import before tile)
import concourse.mybir as mybir
import concourse.tile as tile
from concourse import bacc
from concourse.bass_utils import run_bass_kernel_spmd
from concourse.masks import make_identity

B, H, S, D = 2, 16, 2048, 64
N_CORES = 8
HPC = (B * H) // N_CORES  # heads per core = 4

F32 = mybir.dt.float32
F16 = mybir.dt.float16
I16 = mybir.dt.int16
EXP = mybir.ActivationFunctionType.Exp

SCALE = 1.0 / 8.0  # 1/sqrt(D)
NQ = S // 128      # 16 tiles of 128 along both q and k
QH = 512           # q-columns per pass
NP = NQ // 2       # 8 score pairs per pass

# score pairs (of 8 per pass) whose exp runs on the VectorEngine bit-trick
OFF_PAIRS = (1, 4, 6)
EXP_A = math.log2(math.e) * 1024.0 / 8.0          # 184.665 (includes 1/sqrt(D))
EXP_B = 15.0 * 1024.0 - 59.29                     # mean-centering constant

# head-0 ramp transposes emitted inside pass iterations: (pass_idx, iter) ->
# list of ("q"|"k", tile).  Initial burst before pass 0: k0-5, q0-3.
RAMP_TP = {
    (0, 0): [("k", 6), ("k", 7)],
    (0, 1): [("k", 8), ("k", 9)],
    (0, 2): [("k", 10), ("k", 11)],
    (0, 3): [("k", 12), ("k", 13), ("q", 4)],
    (0, 4): [("k", 14), ("k", 15), ("q", 5)],
    (0, 5): [("q", 6)],
    (0, 6): [("q", 7)],
    (1, 0): [("q", 8), ("q", 9)],
    (1, 1): [("q", 10), ("q", 11)],
    (1, 2): [("q", 12), ("q", 13)],
    (1, 3): [("q", 14), ("q", 15)],
}


class _HeadInputs:
    """Per-head staged inputs: fp16 Q^T/K^T [128, S] (rows 0..63 data, rows
    64..127 a DUPLICATE of the data so row-tiled pair matmuls can read both
    halves) and [V | 1].

    Heads 1-3: transposes run on the DMA X-bar through a DRAM bounce: fp32
    load -> fp16 cast -> two half-column DRAM stores (data + duplicate) ->
    one transposed load.  Zero PE cost, same DMA bytes as a zero-padded
    bounce."""

    def __init__(self, ctx, h):
        self.ctx = ctx
        self.h = h

    def emit_transpose(self, kind, n):
        """Head-0 ramp transpose on the TensorEngine.  q16/k16 tiles carry
        the duplicate in columns 64-127, so one full [128, 128] transpose
        lands data + duplicate rows in one shot.  PSUM is fully booked, so
        the output borrows an f16 bitcast view of a scores pair slot."""
        nc, pools = self.ctx["nc"], self.ctx
        st16, tT = (self.q16, self.qT) if kind == "q" else (self.k16, self.kT)
        host = pools["ps_s_pool"].tile([128, QH * 2], F32, tag="s", name=f"tp_{kind}{n}")
        tp = host[:, :64].bitcast(F16)
        nc.tensor.transpose(tp, st16[:, n, :], pools["ident16"][:])
        nc.vector.tensor_copy(tT[:, n * 128:(n + 1) * 128], tp)

    def start_dma_split(self):
        """Head-0 ramp: PE is idle, so transpose on the TensorEngine
        (shorter critical chain than the DRAM bounce).  Casts write BOTH
        column halves (data + duplicate) so the transposes are full-tile."""
        nc, pools, h = self.ctx["nc"], self.ctx, self.h
        head_pool = pools["head_pool"]
        qkt_pool = pools["qkt_pool"]

        q_nat = head_pool.tile([128, NQ, D], F32, tag="q_nat", name=f"q_nat{h}")
        k_nat = head_pool.tile([128, NQ, D], F32, tag="k_nat", name=f"k_nat{h}")
        v_nat = head_pool.tile([128, NQ, D], F32, tag="v_nat", name=f"v_nat{h}")
        q16 = head_pool.tile([128, NQ, 128], F16, tag="q16", name=f"q16_{h}")
        k16 = head_pool.tile([128, NQ, 128], F16, tag="k16", name=f"k16_{h}")
        self.q16, self.k16 = q16, k16
        self.qT = qkt_pool.tile([128, S], F16, tag="qT", name=f"qT{h}")
        self.kT = qkt_pool.tile([128, S], F16, tag="kT", name=f"kT{h}")
        qd = pools["q_dram"][h].rearrange("(n p) d -> p n d", p=128)
        kd = pools["k_dram"][h].rearrange("(n p) d -> p n d", p=128)
        chunks = ((k_nat, k16, kd, 0, 6), (q_nat, q16, qd, 0, 4),
                  (k_nat, k16, kd, 6, NQ), (q_nat, q16, qd, 4, NQ))
        for ci, (nat, st16, dr, a, b) in enumerate(chunks):
            nc.sync.dma_start(nat[:, a:b, :], dr[:, a:b, :])
            nc.vector.tensor_copy(st16[:, a:b, :D], nat[:, a:b, :])
            nc.vector.tensor_copy(st16[:, a:b, D:], nat[:, a:b, :])
            if ci == 1:
                # V rides after the first critical chunks; PV k=0 needs it
                # only once exp of the first pair lands
                nc.sync.dma_start(
                    v_nat[:],
                    pools["v_dram"][h].rearrange("(n p) d -> p n d", p=128),
                )
        # transposes that unblock the first few score pairs; the rest are
        # emitted inside pass-0/1 iterations (RAMP_TP)
        for kind, n in (("k", 0), ("k", 1), ("q", 0), ("q", 1), ("q", 2),
                        ("q", 3), ("k", 2), ("k", 3), ("k", 4), ("k", 5)):
            self.emit_transpose(kind, n)
        v1 = head_pool.tile([128, NQ, D + 1], F16, tag="v1", name=f"v1_{h}")
        nc.vector.tensor_copy(
            v1[:, :, D:].rearrange("p n one -> p (n one)"),
            pools["ones"][:],
        )
        nc.vector.tensor_copy(v1[:, :, :D], v_nat[:])
        self.v1 = v1

    def stage(self, i):
        """Emit staging stage i (0..5).  The stages are spread across passes
        so every SP DMA trigger's producer (the slow GpSimd casts above all)
        has completed by the time the in-order SP queue reaches it:
          0: q load + q casts   1: k load + k casts   2: v load + q store
          3: v1 build           4: k store + q X-bar  5: k X-bar
        Casts write BOTH column halves (data + duplicate) on the
        otherwise-idle GpSimd engine, a full head ahead of use."""
        nc, pools, h = self.ctx["nc"], self.ctx, self.h
        head_pool = pools["head_pool"]
        dram_pool = pools["dram_pool"]
        qkt_pool = pools["qkt_pool"]

        # head 1 is ramp-critical: its lo-half casts + v1 run on the (then
        # mostly idle) ScalarEngine concurrently with GpSimd's hi-half casts
        fast = nc.scalar.copy if self.h == 1 else nc.gpsimd.tensor_copy
        if i == 0:
            self.q_nat = head_pool.tile([128, NQ, D], F32, tag="q_nat", name=f"q_nat{h}")
            self.k_nat = head_pool.tile([128, NQ, D], F32, tag="k_nat", name=f"k_nat{h}")
            self.v_nat = head_pool.tile([128, NQ, D], F32, tag="v_nat", name=f"v_nat{h}")
            self.q16 = head_pool.tile([128, NQ, 128], F16, tag="q16", name=f"q16_{h}")
            self.k16 = head_pool.tile([128, NQ, 128], F16, tag="k16", name=f"k16_{h}")
            self.qdr = dram_pool.tile([S, 128], F16, tag="qdr", name=f"qdr{h}")
            self.kdr = dram_pool.tile([S, 128], F16, tag="kdr", name=f"kdr{h}")
            self.qT = qkt_pool.tile([128, S], F16, tag="qT", name=f"qT{h}")
            self.kT = qkt_pool.tile([128, S], F16, tag="kT", name=f"kT{h}")
            nc.sync.dma_start(
                self.q_nat[:],
                pools["q_dram"][h].rearrange("(n p) d -> p n d", p=128),
            )
            fast(self.q16[:, :, :D], self.q_nat[:])
            nc.gpsimd.tensor_copy(self.q16[:, :, D:], self.q_nat[:])
        elif i == 1:
            nc.sync.dma_start(
                self.k_nat[:],
                pools["k_dram"][h].rearrange("(n p) d -> p n d", p=128),
            )
            fast(self.k16[:, :, :D], self.k_nat[:])
            nc.gpsimd.tensor_copy(self.k16[:, :, D:], self.k_nat[:])
        elif i == 2:
            nc.sync.dma_start(
                self.v_nat[:],
                pools["v_dram"][h].rearrange("(n p) d -> p n d", p=128),
            )
            nc.sync.dma_start(
                self.qdr[:].rearrange("(n p) c -> p n c", p=128), self.q16[:]
            )
        elif i == 3:
            v1 = head_pool.tile([128, NQ, D + 1], F16, tag="v1", name=f"v1_{h}")
            fast(
                v1[:, :, D:].rearrange("p n one -> p (n one)"), pools["ones"][:]
            )
            fast(v1[:, :, :D], self.v_nat[:])
            self.v1 = v1
        elif i == 4:
            nc.sync.dma_start(
                self.kdr[:].rearrange("(n p) c -> p n c", p=128), self.k16[:]
            )
            nc.sync.dma_start_transpose(self.qT[:], self.qdr[:])
        elif i == 5:
            nc.sync.dma_start_transpose(self.kT[:], self.kdr[:])


def _attention(tc):
    nc = tc.nc
    q_dram = nc.dram_tensor("query", [HPC, S, D], F32, kind="ExternalInput").ap()
    k_dram = nc.dram_tensor("key", [HPC, S, D], F32, kind="ExternalInput").ap()
    v_dram = nc.dram_tensor("value", [HPC, S, D], F32, kind="ExternalInput").ap()
    o_dram = nc.dram_tensor("out", [HPC, S, D], F32, kind="ExternalOutput").ap()

    with (
        tc.tile_pool(name="const", bufs=1) as const_pool,
        tc.tile_pool(name="head_io", bufs=3) as head_pool,
        tc.tile_pool(name="qkt", bufs=3) as qkt_pool,
        tc.tile_pool(name="et", bufs=4) as et_pool,
        tc.tile_pool(name="eti", bufs=2) as eti_pool,
        tc.tile_pool(name="epi", bufs=2) as epi_pool,
        tc.tile_pool(name="dram", bufs=2, space="DRAM") as dram_pool,
        tc.tile_pool(name="ps_s", bufs=3, space="PSUM") as ps_s_pool,
        tc.tile_pool(name="ps_o", bufs=2, space="PSUM") as ps_o_pool,
    ):
        ident16 = const_pool.tile([128, 128], F16)
        make_identity(nc, ident16[:])
        ones = const_pool.tile([128, NQ], F16)
        nc.vector.memset(ones[:], 1.0)
        wsrc = const_pool.tile([128, 512], F16)
        nc.vector.memset(wsrc[:], 0.015625)
        # [1, 65] unit row (1 at col 64) and [1, 512] ones: a K=1 matmul of
        # these inside the PV accumulation adds the restricted softmax's +1
        # to the denominator row of oT for free
        e65 = const_pool.tile([1, D + 1], F16)
        nc.vector.memset(e65[:], 0.0)
        nc.vector.memset(e65[:, D:], 1.0)
        ones512 = const_pool.tile([1, QH], F16)
        nc.vector.memset(ones512[:], 1.0)

        # ~10 dummy matmuls warm the PE's HAM clock gate (~3.4us of PE busy
        # flips it to 2.4 GHz) while the first DMAs land
        warm_ps = ps_s_pool.tile([128, QH * 2], F32, tag="s", name="warm")
        for i in range(10):
            nc.tensor.matmul(
                warm_ps[:, :512], wsrc[:, :128], wsrc[:],
                start=True, stop=True,
            )

        ctx = {
            "nc": nc, "q_dram": q_dram, "k_dram": k_dram, "v_dram": v_dram,
            "head_pool": head_pool, "qkt_pool": qkt_pool,
            "ps_s_pool": ps_s_pool, "dram_pool": dram_pool,
            "ident16": ident16, "ones": ones,
        }

        heads = [_HeadInputs(ctx, h) for h in range(HPC)]
        heads[0].start_dma_split()

        def emit_pair(hd, qh, p, sp):
            """Two k-tiles (2p, 2p+1) of scores, concurrently as K=64
            row-tiles at tile_position (0,0) / (64,0)."""
            q0 = qh * QH
            j = 2 * p
            nc.tensor.matmul(
                sp[:, 0:QH],
                hd.kT[0:64, j * 128:(j + 1) * 128],
                hd.qT[0:64, q0:q0 + QH],
                start=True, stop=True,
            )
            nc.tensor.matmul(
                sp[:, QH:2 * QH],
                hd.kT[64:128, (j + 1) * 128:(j + 2) * 128],
                hd.qT[64:128, q0:q0 + QH],
                start=True, stop=True,
            )

        def emit_pv(hd, oT, p, et_ap):
            for j in (0, 1):
                k = 2 * p + j
                nc.tensor.matmul(
                    oT[:D + 1, :],
                    hd.v1[:, k, :],
                    et_ap[:, j * QH:(j + 1) * QH],
                    start=(k == 0), stop=(k == NQ - 1),
                )
                if k == 0:
                    # denominator += 1 (rank-1: e65.T @ ones512 hits row 64)
                    nc.tensor.matmul(
                        oT[:D + 1, :], e65[:], ones512[:],
                        start=False, stop=False,
                    )

        def epi_copy(st, half=None):
            """Stage A: evict oT PSUM -> fp16 SBUF (DVE), and drop the fp16
            reciprocal of the denominator into row 65 so the X-bar transpose
            delivers it per-q.  Depends only on PSUM -- never on a DMA."""
            if "oT16" not in st:
                st["oT16"] = epi_pool.tile([80, QH], F16, tag="oT16", name="oT16")
            hs = slice(None) if half is None else slice(half * (QH // 2), (half + 1) * (QH // 2))
            nc.vector.tensor_copy(st["oT16"][:D + 1, hs], st["oT"][:D + 1, hs])
            # in-place fp16 reciprocal of the denominator row (partition 64,
            # 32-aligned; engine APs cannot start at unaligned partitions)
            with nc.allow_low_precision(reason="fp16 recip: 5e-4 rel, gate is 2e-2"):
                nc.vector.reciprocal(st["oT16"][D:D + 1, hs], st["oT16"][D:D + 1, hs])

        def epi_transpose(st, half=None):
            """Stage B: X-bar transpose [80, cols] -> [128, cols//128, 80]."""
            if "trT" not in st:
                st["trT"] = epi_pool.tile([128, QH // 128, 80], F16, tag="trT", name="trT")
            nb = QH // 128
            bs = slice(None) if half is None else slice(half * (nb // 2), (half + 1) * (nb // 2))
            hs = slice(None) if half is None else slice(half * (QH // 2), (half + 1) * (QH // 2))
            nc.sync.dma_start_transpose(st["trT"][:, bs, :], st["oT16"][:, hs])

        def epi_normalize(st, half=None):
            """Stage C: one GpSimd multiply by the transposed reciprocal
            (trT row 65) + out DMA.  On GpSimd because it waits on the trT
            transpose DMA: a DMA-waiting op on the DVE queue would block the
            etis behind it and starve the PE's PV matmuls; on GpSimd the
            only things queued behind are slack-rich casts."""
            h, qh, trT = st["h"], st["qh"], st["trT"]
            nb = QH // 128
            bs = range(nb) if half is None else range(half * nb // 2, (half + 1) * nb // 2)
            if "o_sb" not in st:
                st["o_sb"] = epi_pool.tile([128, nb, D], F32, tag="o_sb", name="o_sb")
            o_sb = st["o_sb"]
            j0, j1 = min(bs), max(bs) + 1
            nc.gpsimd.tensor_tensor(
                o_sb[:, j0:j1, :], trT[:, j0:j1, :D],
                trT[:, j0:j1, D].broadcast_to([128, j1 - j0, D]),
                mybir.AluOpType.mult,
            )
            nc.sync.dma_start(
                o_dram[h].rearrange("(n p) d -> p n d", p=128)[:, qh * nb + j0:qh * nb + j1, :],
                o_sb[:, j0:j1, :],
            )

        # staging stage schedule: (pass_idx -> [(head, stage)]).  Spaced so
        # every SP trigger's producer (slow GpSimd casts, DMA transfers) has
        # completed by the time the in-order SP queue reaches it, and each
        # head's X-bar transposes land >=1 pass before its first use.
        stage_at = {}
        for hh in range(1, HPC):
            pl = {1: [0, 0, 1, 1, 2, 2],
                  2: [1, 2, 3, 4, 4, 5],
                  3: [4, 5, 6, 7, 8, 9]}[hh]
            for si, pp in enumerate(pl):
                stage_at.setdefault(pp, []).append((hh, si))

        passes = [(h, qh) for h in range(HPC) for qh in range(S // QH)]
        NG = len(passes) * NP  # 128 global pair iterations

        def new_pair(g):
            """Allocate + emit score pair g (global index)."""
            h2, qh2 = passes[g // NP]
            sp = ps_s_pool.tile([128, QH * 2], F32, tag="s", name=f"sp{g}")
            emit_pair(heads[h2], qh2, g % NP, sp)
            return sp

        ets = {}      # g -> et AP awaiting its (lagged) PV
        sps = {}      # g -> produced pair tile
        epi_st = {}   # pass idx -> epilogue state
        oT = None

        def emit_pv_g(g):
            """PV for pair g, lagged one iteration behind its exp so the
            PE's in-order queue never stalls waiting for an exp result."""
            nonlocal oT
            idx2, p2 = g // NP, g % NP
            if p2 == 0:
                oT = ps_o_pool.tile([D + 1, QH], F32, tag="oT", name="oT")
                epi_st[idx2] = {"h": passes[idx2][0], "qh": passes[idx2][1], "oT": oT}
            emit_pv(heads[passes[idx2][0]], oT, p2, ets.pop(g))

        # score pairs are produced TWO iterations ahead of their exp (pairs
        # only one ahead get trapped behind PE stalls and starve the exp
        # engines); 3 PSUM pair slots = depth 3 (producing/waiting/exp'ing) --
        # the slot frees at exp time, so the lagged PV costs no extra slot.
        sps[0] = new_pair(0)
        sps[1] = new_pair(1)
        for g in range(NG):
            idx2, p2 = g // NP, g % NP
            if p2 == 0:
                for hh, si in stage_at.get(idx2, ()):
                    heads[hh].stage(si)
            for kind, n in RAMP_TP.get((idx2, p2), ()):
                heads[0].emit_transpose(kind, n)
            sp = sps.pop(g)
            if p2 in OFF_PAIRS:
                eti = eti_pool.tile([128, QH * 2], I16, tag="eti", name=f"eti{g}")
                nc.vector.tensor_scalar(
                    eti[:], sp[:], EXP_A, EXP_B,
                    mybir.AluOpType.mult, mybir.AluOpType.add,
                )
                ets[g] = eti[:].bitcast(F16)
            else:
                et = et_pool.tile([128, QH * 2], F16, tag="et", name=f"et{g}")
                nc.scalar.activation(et[:], sp[:], EXP, scale=SCALE)
                ets[g] = et[:]
            if g + 2 < NG:
                sps[g + 2] = new_pair(g + 2)
            # epilogue stage drains, placed so (a) the DVE copy+recip sits
            # AFTER all of this pass's etis in the DVE queue, (b) each DMA
            # trigger's dependency is long met when the in-order SP queue
            # reaches it
            if p2 == 7 and idx2 - 1 in epi_st:
                epi_copy(epi_st[idx2 - 1])
            elif p2 == 1 and idx2 - 2 in epi_st:
                epi_transpose(epi_st[idx2 - 2])
            elif p2 == 5 and idx2 - 2 in epi_st:
                epi_normalize(epi_st.pop(idx2 - 2))
            if g >= 1:
                emit_pv_g(g - 1)
        emit_pv_g(NG - 1)
        # tail: pipeline the final epilogues in q-halves so the DVE copy,
        # X-bar transpose and normalize overlap instead of chaining serially
        last = len(passes) - 1
        epi_copy(epi_st[last], half=0)
        epi_transpose(epi_st[last - 1])
        epi_copy(epi_st[last], half=1)
        epi_normalize(epi_st.pop(last - 1))
        epi_transpose(epi_st[last], half=0)
        epi_transpose(epi_st[last], half=1)
        epi_normalize(epi_st[last], half=0)
        epi_normalize(epi_st.pop(last), half=1)


_NC_CACHE = None
_TRACE_READY = False


def _enable_tracing():
    """Register the NTFF profile hook that this image's antenv lacks, and
    keep profiling artifacts local instead of uploading to a bucket."""
    global _TRACE_READY
    if _TRACE_READY:
        return
    import sys
    import types

    import antenv
    import concourse.bass_utils as bu
    from trn_agent_boot.trn_boot import _ntff_profile_via_ctypes

    if "antenv.axon_hooks" not in sys.modules:
        mod = types.ModuleType("antenv.axon_hooks")
        mod._hook = None

        def set_axon_ntff_profile_hook(h):
            mod._hook = h

        def get_axon_ntff_profile_hook():
            return mod._hook

        mod.set_axon_ntff_profile_hook = set_axon_ntff_profile_hook
        mod.get_axon_ntff_profile_hook = get_axon_ntff_profile_hook
        sys.modules["antenv.axon_hooks"] = mod
        antenv.axon_hooks = mod

    hooks = sys.modules["antenv.axon_hooks"]
    if hooks.get_axon_ntff_profile_hook() is None:
        hooks.set_axon_ntff_profile_hook(
            _ntff_profile_via_ctypes("/opt/axon/libaxon_pjrt.so")
        )
    bu.upload_artifacts = lambda tmpdir: tmpdir
    _TRACE_READY = True


def _build():
    global _NC_CACHE
    if _NC_CACHE is None:
        nc = bacc.Bacc("TRN2", target_bir_lowering=False, debug=False)
        with tile.TileContext(nc) as tc:
            _attention(tc)
        nc.compile()
        _NC_CACHE = nc
    return _NC_CACHE


def _run(query, key, value, trace=False, tmpdir=None):
    if trace:
        _enable_tracing()
    q = np.ascontiguousarray(np.asarray(query, dtype=np.float32).reshape(B * H, S, D))
    k = np.ascontiguousarray(np.asarray(key, dtype=np.float32).reshape(B * H, S, D))
    v = np.ascontiguousarray(np.asarray(value, dtype=np.float32).reshape(B * H, S, D))
    in_maps = [
        {
            "query": q[c * HPC:(c + 1) * HPC],
            "key": k[c * HPC:(c + 1) * HPC],
            "value": v[c * HPC:(c + 1) * HPC],
        }
        for c in range(N_CORES)
    ]
    nc = _build()
    res = run_bass_kernel_spmd(
        nc, in_maps, core_ids=list(range(N_CORES)), trace=trace, tmpdir=tmpdir
    )
    out = np.stack([res.results[c]["out"] for c in range(N_CORES)])  # [8, HPC, S, D]
    return out.reshape(B, H, S, D), res


def kernel(query, key, value):
    out, _ = _run(query, key, value, trace=bool(int(os.environ.get("BASS_TRACE", "0"))))
    return out


# revision 31
# speedup vs baseline: 1.1622x; 1.0864x over previous
"""Multi-head attention with "restricted softmax" on 8 TRN2 NeuronCores.

Reference computation (per head):
    score = Q @ K.T / sqrt(D)                       # [S, S]
    attn  = exp(score) / (1 + sum_k exp(score))     # restricted softmax
    out   = attn @ V                                # [S, D]

Full problem: B=2, H=16, S=2048, D=64  ->  32 heads, 4 heads per core.

Per-core kernel strategy (no communication needed):
  - Scores computed TRANSPOSED (S^T[k, q]) so softmax's k-reduction sits on
    the PSUM partition axis where the PE performs it for free: PV uses
    lhsT=[V | 1] so the extra output row is sum_k exp = the denominator.
  - The scores contraction is only d=64, so TWO k-tiles run CONCURRENTLY as
    row-tiled K=64 matmuls (tile_position (0,0) / (64,0)): qT/kT rows 64-127
    hold a DUPLICATE of rows 0-63 (instead of zero padding) so the second
    row-group has data to read.  Measured 106 ns per N=512 matmul in pairs
    vs 216 solo -- the scores phase runs at 2 cols/cycle.
  - Passes are QH=512 q-columns wide; a score pair lands in ONE [128, 1024]
    PSUM tile (2 banks), so the ScalarEngine exps a whole pair per ACTIVATE
    (halves the 352-cycle per-call overhead).  PSUM: 3 pair slots (6 banks)
    + double-buffered oT [65, 512] (2 banks) = 8.
  - exp split per pass: 5 pairs on ScalarE, 3 pairs on the VectorEngine via
    a Schraudolph-style fp16 bit-pattern exp (i16 = s*(log2e*1024/8) +
    (15*1024 - 59.3), bitcast to fp16 == exp(score/8), +-2% mantissa ripple).
  - ~10 dummy matmuls on garbage data at kernel start warm the PE's HAM
    clock gate during the DMA ramp, so the real matmul stream never runs
    at the cold 1.2 GHz clock.
  - Q/K transposes for heads 1-3 ride the DMA X-bar via a DRAM bounce
    (fp16 [S, 128] with both column halves = the data).  Head 0 uses
    TensorEngine transposes of duplicated-half [128, 128] tiles, emitted
    interleaved with pass-0/1 iterations so they never serialize ahead of
    the scores stream.
  - Epilogue per pass: evict oT to fp16, X-bar transpose back to [q, d],
    normalize (reciprocal on DVE, the per-block multiplies on the
    otherwise-idle GpSimd engine), DMA out.  The final pass's epilogue is
    split in halves so its stages pipeline instead of chaining serially.
"""

import math
import os

import numpy as np

import concourse.bass as bass  # noqa: F401  (bass must import before tile)
import concourse.mybir as mybir
import concourse.tile as tile
from concourse import bacc
from concourse.bass_utils import run_bass_kernel_spmd
from concourse.masks import make_identity

B, H, S, D = 2, 16, 2048, 64
N_CORES = 8
HPC = (B * H) // N_CORES  # heads per core = 4

F32 = mybir.dt.float32
F16 = mybir.dt.float16
I16 = mybir.dt.int16
EXP = mybir.ActivationFunctionType.Exp

SCALE = 1.0 / 8.0  # 1/sqrt(D)
NQ = S // 128      # 16 tiles of 128 along both q and k
QH = 512           # q-columns per pass
NP = NQ // 2       # 8 score pairs per pass

# score pairs (of 8 per pass) whose exp runs on the VectorEngine bit-trick
OFF_PAIRS = (1, 4, 6)
EXP_A = math.log2(math.e) * 1024.0 / 8.0          # 184.665 (includes 1/sqrt(D))
EXP_B = 15.0 * 1024.0 - 59.29                     # mean-centering constant

# head-0 ramp transposes emitted inside pass iterations: (pass_idx, iter) ->
# list of ("q"|"k", tile).  Initial burst before pass 0: k0-5, q0-3.
RAMP_TP = {
    (0, 0): [("k", 6), ("k", 7)],
    (0, 1): [("k", 8), ("k", 9)],
    (0, 2): [("k", 10), ("k", 11)],
    (0, 3): [("k", 12), ("k", 13), ("q", 4)],
    (0, 4): [("k", 14), ("k", 15), ("q", 5)],
    (0, 5): [("q", 6)],
    (0, 6): [("q", 7)],
    (1, 0): [("q", 8), ("q", 9)],
    (1, 1): [("q", 10), ("q", 11)],
    (1, 2): [("q", 12), ("q", 13)],
    (1, 3): [("q", 14), ("q", 15)],
}


class _HeadInputs:
    """Per-head staged inputs: fp16 Q^T/K^T [128, S] (rows 0..63 data, rows
    64..127 a DUPLICATE of the data so row-tiled pair matmuls can read both
    halves) and [V | 1].

    Heads 1-3: transposes run on the DMA X-bar through a DRAM bounce: fp32
    load -> fp16 cast -> two half-column DRAM stores (data + duplicate) ->
    one transposed load.  Zero PE cost, same DMA bytes as a zero-padded
    bounce."""

    def __init__(self, ctx, h):
        self.ctx = ctx
        self.h = h

    def emit_transpose(self, kind, n):
        """Head-0 ramp transpose on the TensorEngine.  q16/k16 tiles carry
        the duplicate in columns 64-127, so one full [128, 128] transpose
        lands data + duplicate rows in one shot.  PSUM is fully booked, so
        the output borrows an f16 bitcast view of a scores pair slot."""
        nc, pools = self.ctx["nc"], self.ctx
        st16, tT = (self.q16, self.qT) if kind == "q" else (self.k16, self.kT)
        host = pools["ps_s_pool"].tile([128, QH * 2], F32, tag="s", name=f"tp_{kind}{n}")
        tp = host[:, :64].bitcast(F16)
        nc.tensor.transpose(tp, st16[:, n, :], pools["ident16"][:])
        nc.vector.tensor_copy(tT[:, n * 128:(n + 1) * 128], tp)

    def start_dma_split(self):
        """Head-0 ramp: PE is idle, so transpose on the TensorEngine
        (shorter critical chain than the DRAM bounce).  Casts write BOTH
        column halves (data + duplicate) so the transposes are full-tile."""
        nc, pools, h = self.ctx["nc"], self.ctx, self.h
        head_pool = pools["head_pool"]
        qkt_pool = pools["qkt_pool"]

        q_nat = head_pool.tile([128, NQ, D], F32, tag="q_nat", name=f"q_nat{h}")
        k_nat = head_pool.tile([128, NQ, D], F32, tag="k_nat", name=f"k_nat{h}")
        v_nat = head_pool.tile([128, NQ, D], F32, tag="v_nat", name=f"v_nat{h}")
        q16 = head_pool.tile([128, NQ, 128], F16, tag="q16", name=f"q16_{h}")
        k16 = head_pool.tile([128, NQ, 128], F16, tag="k16", name=f"k16_{h}")
        self.q16, self.k16 = q16, k16
        self.qT = qkt_pool.tile([128, S], F16, tag="qT", name=f"qT{h}")
        self.kT = qkt_pool.tile([128, S], F16, tag="kT", name=f"kT{h}")
        qd = pools["q_dram"][h].rearrange("(n p) d -> p n d", p=128)
        kd = pools["k_dram"][h].rearrange("(n p) d -> p n d", p=128)
        chunks = ((k_nat, k16, kd, 0, 6), (q_nat, q16, qd, 0, 4),
                  (k_nat, k16, kd, 6, NQ), (q_nat, q16, qd, 4, NQ))
        for ci, (nat, st16, dr, a, b) in enumerate(chunks):
            nc.sync.dma_start(nat[:, a:b, :], dr[:, a:b, :])
            nc.vector.tensor_copy(st16[:, a:b, :D], nat[:, a:b, :])
            nc.vector.tensor_copy(st16[:, a:b, D:], nat[:, a:b, :])
            if ci == 1:
                # V rides after the first critical chunks; PV k=0 needs it
                # only once exp of the first pair lands
                nc.sync.dma_start(
                    v_nat[:],
                    pools["v_dram"][h].rearrange("(n p) d -> p n d", p=128),
                )
        # transposes that unblock the first few score pairs; the rest are
        # emitted inside pass-0/1 iterations (RAMP_TP)
        for kind, n in (("k", 0), ("k", 1), ("q", 0), ("q", 1), ("q", 2),
                        ("q", 3), ("k", 2), ("k", 3), ("k", 4), ("k", 5)):
            self.emit_transpose(kind, n)
        v1 = head_pool.tile([128, NQ, D + 1], F16, tag="v1", name=f"v1_{h}")
        nc.vector.tensor_copy(
            v1[:, :, D:].rearrange("p n one -> p (n one)"),
            pools["ones"][:],
        )
        nc.vector.tensor_copy(v1[:, :, :D], v_nat[:])
        self.v1 = v1

    def stage(self, i):
        """Emit staging stage i (0..5).  The stages are spread across passes
        so every SP DMA trigger's producer (the slow GpSimd casts above all)
        has completed by the time the in-order SP queue reaches it:
          0: q load + q casts   1: k load + k casts   2: v load + q store
          3: v1 build           4: k store + q X-bar  5: k X-bar
        Casts write BOTH column halves (data + duplicate) on the
        otherwise-idle GpSimd engine, a full head ahead of use."""
        nc, pools, h = self.ctx["nc"], self.ctx, self.h
        head_pool = pools["head_pool"]
        dram_pool = pools["dram_pool"]
        qkt_pool = pools["qkt_pool"]

        # head 1 is ramp-critical: its lo-half casts + v1 run on the (then
        # mostly idle) ScalarEngine concurrently with GpSimd's hi-half casts
        fast = nc.scalar.copy if self.h == 1 else nc.gpsimd.tensor_copy
        if i == 0:
            self.q_nat = head_pool.tile([128, NQ, D], F32, tag="q_nat", name=f"q_nat{h}")
            self.k_nat = head_pool.tile([128, NQ, D], F32, tag="k_nat", name=f"k_nat{h}")
            self.v_nat = head_pool.tile([128, NQ, D], F32, tag="v_nat", name=f"v_nat{h}")
            self.q16 = head_pool.tile([128, NQ, 128], F16, tag="q16", name=f"q16_{h}")
            self.k16 = head_pool.tile([128, NQ, 128], F16, tag="k16", name=f"k16_{h}")
            self.qdr = dram_pool.tile([S, 128], F16, tag="qdr", name=f"qdr{h}")
            self.kdr = dram_pool.tile([S, 128], F16, tag="kdr", name=f"kdr{h}")
            self.qT = qkt_pool.tile([128, S], F16, tag="qT", name=f"qT{h}")
            self.kT = qkt_pool.tile([128, S], F16, tag="kT", name=f"kT{h}")
            nc.sync.dma_start(
                self.q_nat[:],
                pools["q_dram"][h].rearrange("(n p) d -> p n d", p=128),
            )
            fast(self.q16[:, :, :D], self.q_nat[:])
            nc.gpsimd.tensor_copy(self.q16[:, :, D:], self.q_nat[:])
        elif i == 1:
            nc.sync.dma_start(
                self.k_nat[:],
                pools["k_dram"][h].rearrange("(n p) d -> p n d", p=128),
            )
            fast(self.k16[:, :, :D], self.k_nat[:])
            nc.gpsimd.tensor_copy(self.k16[:, :, D:], self.k_nat[:])
        elif i == 2:
            nc.sync.dma_start(
                self.v_nat[:],
                pools["v_dram"][h].rearrange("(n p) d -> p n d", p=128),
            )
            nc.sync.dma_start(
                self.qdr[:].rearrange("(n p) c -> p n c", p=128), self.q16[:]
            )
        elif i == 3:
            v1 = head_pool.tile([128, NQ, D + 1], F16, tag="v1", name=f"v1_{h}")
            fast(
                v1[:, :, D:].rearrange("p n one -> p (n one)"), pools["ones"][:]
            )
            fast(v1[:, :, :D], self.v_nat[:])
            self.v1 = v1
        elif i == 4:
            nc.sync.dma_start(
                self.kdr[:].rearrange("(n p) c -> p n c", p=128), self.k16[:]
            )
            nc.sync.dma_start_transpose(self.qT[:], self.qdr[:])
        elif i == 5:
            nc.sync.dma_start_transpose(self.kT[:], self.kdr[:])


def _attention(tc):
    nc = tc.nc
    q_dram = nc.dram_tensor("query", [HPC, S, D], F32, kind="ExternalInput").ap()
    k_dram = nc.dram_tensor("key", [HPC, S, D], F32, kind="ExternalInput").ap()
    v_dram = nc.dram_tensor("value", [HPC, S, D], F32, kind="ExternalInput").ap()
    o_dram = nc.dram_tensor("out", [HPC, S, D], F32, kind="ExternalOutput").ap()

    with (
        tc.tile_pool(name="const", bufs=1) as const_pool,
        tc.tile_pool(name="head_io", bufs=3) as head_pool,
        tc.tile_pool(name="qkt", bufs=3) as qkt_pool,
        tc.tile_pool(name="et", bufs=4) as et_pool,
        tc.tile_pool(name="eti", bufs=2) as eti_pool,
        tc.tile_pool(name="epi", bufs=2) as epi_pool,
        tc.tile_pool(name="dram", bufs=2, space="DRAM") as dram_pool,
        tc.tile_pool(name="ps_s", bufs=3, space="PSUM") as ps_s_pool,
        tc.tile_pool(name="ps_o", bufs=2, space="PSUM") as ps_o_pool,
    ):
        ident16 = const_pool.tile([128, 128], F16)
        make_identity(nc, ident16[:])
        ones = const_pool.tile([128, NQ], F16)
        nc.vector.memset(ones[:], 1.0)
        wsrc = const_pool.tile([128, 512], F16)
        nc.vector.memset(wsrc[:], 0.015625)
        # [1, 65] unit row (1 at col 64) and [1, 512] ones: a K=1 matmul of
        # these inside the PV accumulation adds the restricted softmax's +1
        # to the denominator row of oT for free
        e65 = const_pool.tile([1, D + 1], F16)
        nc.vector.memset(e65[:], 0.0)
        nc.vector.memset(e65[:, D:], 1.0)
        ones512 = const_pool.tile([1, QH], F16)
        nc.vector.memset(ones512[:], 1.0)

        # ~10 dummy matmuls warm the PE's HAM clock gate (~3.4us of PE busy
        # flips it to 2.4 GHz) while the first DMAs land
        warm_ps = ps_s_pool.tile([128, QH * 2], F32, tag="s", name="warm")
        for i in range(10):
            nc.tensor.matmul(
                warm_ps[:, :512], wsrc[:, :128], wsrc[:],
                start=True, stop=True,
            )

        ctx = {
            "nc": nc, "q_dram": q_dram, "k_dram": k_dram, "v_dram": v_dram,
            "head_pool": head_pool, "qkt_pool": qkt_pool,
            "ps_s_pool": ps_s_pool, "dram_pool": dram_pool,
            "ident16": ident16, "ones": ones,
        }

        heads = [_HeadInputs(ctx, h) for h in range(HPC)]
        heads[0].start_dma_split()

        def emit_pair(hd, qh, p, sp):
            """Two k-tiles (2p, 2p+1) of scores, concurrently as K=64
            row-tiles at tile_position (0,0) / (64,0)."""
            q0 = qh * QH
            j = 2 * p
            nc.tensor.matmul(
                sp[:, 0:QH],
                hd.kT[0:64, j * 128:(j + 1) * 128],
                hd.qT[0:64, q0:q0 + QH],
                start=True, stop=True,
            )
            nc.tensor.matmul(
                sp[:, QH:2 * QH],
                hd.kT[64:128, (j + 1) * 128:(j + 2) * 128],
                hd.qT[64:128, q0:q0 + QH],
                start=True, stop=True,
            )

        def emit_pv(hd, oT, p, et_ap):
            for j in (0, 1):
                k = 2 * p + j
                nc.tensor.matmul(
                    oT[:D + 1, :],
                    hd.v1[:, k, :],
                    et_ap[:, j * QH:(j + 1) * QH],
                    start=(k == 0), stop=(k == NQ - 1),
                )
                if k == 0:
                    # denominator += 1 (rank-1: e65.T @ ones512 hits row 64)
                    nc.tensor.matmul(
                        oT[:D + 1, :], e65[:], ones512[:],
                        start=False, stop=False,
                    )

        def epi_copy(st, half=None):
            """Stage A: evict oT PSUM -> fp16 SBUF (DVE), and drop the fp16
            reciprocal of the denominator into row 65 so the X-bar transpose
            delivers it per-q.  Depends only on PSUM -- never on a DMA."""
            if "oT16" not in st:
                st["oT16"] = epi_pool.tile([80, QH], F16, tag="oT16", name="oT16")
            hs = slice(None) if half is None else slice(half * (QH // 2), (half + 1) * (QH // 2))
            nc.vector.tensor_copy(st["oT16"][:D + 1, hs], st["oT"][:D + 1, hs])

        def epi_transpose(st, half=None):
            """Stage B: X-bar transpose [80, cols] -> [128, cols//128, 80]."""
            if "trT" not in st:
                st["trT"] = epi_pool.tile([128, QH // 128, 80], F16, tag="trT", name="trT")
            nb = QH // 128
            bs = slice(None) if half is None else slice(half * (nb // 2), (half + 1) * (nb // 2))
            hs = slice(None) if half is None else slice(half * (QH // 2), (half + 1) * (QH // 2))
            nc.sync.dma_start_transpose(st["trT"][:, bs, :], st["oT16"][:, hs])

        def epi_recip(st, half=None):
            """Stage C: per-q reciprocal of the transposed denominator
            (trT row D already includes the +1 via the rank-1 matmul).
            [128, 4] = 4 elems/lane -- the iterative reciprocal is ~8
            cycles/elem/lane, so this layout is ~150ns (a [1, 512]
            single-lane recip would be 3.3us).  Scheduled 6 iterations
            after the trT transpose trigger and after all of this pass's
            etis on the DVE queue, so it never blocks the PE's PV feed."""
            nb = QH // 128
            if "rec" not in st:
                st["rec"] = epi_pool.tile([128, nb], F32, tag="rec", name="rec")
            bs = range(nb) if half is None else range(half * nb // 2, (half + 1) * nb // 2)
            j0, j1 = min(bs), max(bs) + 1
            nc.vector.reciprocal(st["rec"][:, j0:j1], st["trT"][:, j0:j1, D])

        def epi_normalize(st, half=None):
            """Stage D: one GpSimd multiply by the reciprocal + out DMA.
            On GpSimd because nothing latency-critical queues behind it."""
            h, qh, trT = st["h"], st["qh"], st["trT"]
            nb = QH // 128
            bs = range(nb) if half is None else range(half * nb // 2, (half + 1) * nb // 2)
            if "o_sb" not in st:
                st["o_sb"] = epi_pool.tile([128, nb, D], F32, tag="o_sb", name="o_sb")
            o_sb = st["o_sb"]
            j0, j1 = min(bs), max(bs) + 1
            nc.gpsimd.tensor_tensor(
                o_sb[:, j0:j1, :], trT[:, j0:j1, :D],
                st["rec"][:, j0:j1].broadcast_to([128, j1 - j0, D]),
                mybir.AluOpType.mult,
            )
            nc.sync.dma_start(
                o_dram[h].rearrange("(n p) d -> p n d", p=128)[:, qh * nb + j0:qh * nb + j1, :],
                o_sb[:, j0:j1, :],
            )

        # staging stage schedule: (pass_idx -> [(head, stage)]).  Spaced so
        # every SP trigger's producer (slow GpSimd casts, DMA transfers) has
        # completed by the time the in-order SP queue reaches it, and each
        # head's X-bar transposes land >=1 pass before its first use.
        stage_at = {}
        for hh in range(1, HPC):
            pl = {1: [0, 0, 1, 1, 2, 2],
                  2: [1, 2, 3, 4, 4, 5],
                  3: [4, 5, 6, 7, 8, 9]}[hh]
            for si, pp in enumerate(pl):
                stage_at.setdefault(pp, []).append((hh, si))

        passes = [(h, qh) for h in range(HPC) for qh in range(S // QH)]
        NG = len(passes) * NP  # 128 global pair iterations

        def new_pair(g):
            """Allocate + emit score pair g (global index)."""
            h2, qh2 = passes[g // NP]
            sp = ps_s_pool.tile([128, QH * 2], F32, tag="s", name=f"sp{g}")
            emit_pair(heads[h2], qh2, g % NP, sp)
            return sp

        ets = {}      # g -> et AP awaiting its (lagged) PV
        sps = {}      # g -> produced pair tile
        epi_st = {}   # pass idx -> epilogue state
        oT = None

        def emit_pv_g(g):
            """PV for pair g, lagged one iteration behind its exp so the
            PE's in-order queue never stalls waiting for an exp result."""
            nonlocal oT
            idx2, p2 = g // NP, g % NP
            if p2 == 0:
                oT = ps_o_pool.tile([D + 1, QH], F32, tag="oT", name="oT")
                epi_st[idx2] = {"h": passes[idx2][0], "qh": passes[idx2][1], "oT": oT}
            emit_pv(heads[passes[idx2][0]], oT, p2, ets.pop(g))

        # score pairs are produced TWO iterations ahead of their exp (pairs
        # only one ahead get trapped behind PE stalls and starve the exp
        # engines); 3 PSUM pair slots = depth 3 (producing/waiting/exp'ing) --
        # the slot frees at exp time, so the lagged PV costs no extra slot.
        sps[0] = new_pair(0)
        sps[1] = new_pair(1)
        for g in range(NG):
            idx2, p2 = g // NP, g % NP
            if p2 == 0:
                for hh, si in stage_at.get(idx2, ()):
                    heads[hh].stage(si)
            for kind, n in RAMP_TP.get((idx2, p2), ()):
                heads[0].emit_transpose(kind, n)
            sp = sps.pop(g)
            if p2 in OFF_PAIRS:
                eti = eti_pool.tile([128, QH * 2], I16, tag="eti", name=f"eti{g}")
                nc.vector.tensor_scalar(
                    eti[:], sp[:], EXP_A, EXP_B,
                    mybir.AluOpType.mult, mybir.AluOpType.add,
                )
                ets[g] = eti[:].bitcast(F16)
            else:
                et = et_pool.tile([128, QH * 2], F16, tag="et", name=f"et{g}")
                nc.scalar.activation(et[:], sp[:], EXP, scale=SCALE)
                ets[g] = et[:]
            if g + 2 < NG:
                sps[g + 2] = new_pair(g + 2)
            # epilogue stage drains, placed so (a) DVE epilogue ops sit
            # AFTER all of a pass's etis in the DVE queue with multiple
            # iterations of slack on their DMA dependencies, (b) each DMA
            # trigger's dependency is long met when the in-order SP queue
            # reaches it
            if p2 == 7:
                if idx2 - 1 in epi_st:
                    epi_copy(epi_st[idx2 - 1])
                if idx2 - 2 in epi_st:
                    epi_recip(epi_st[idx2 - 2])
            elif p2 == 1 and idx2 - 2 in epi_st:
                epi_transpose(epi_st[idx2 - 2])
            elif p2 == 2 and idx2 - 3 in epi_st:
                epi_normalize(epi_st.pop(idx2 - 3))
            if g >= 1:
                emit_pv_g(g - 1)
        emit_pv_g(NG - 1)
        # tail: pipeline the final epilogues in q-halves so the DVE copy,
        # X-bar transpose, reciprocal and normalize overlap
        last = len(passes) - 1  # done in-loop: 13: copy/tr/recip, 14: copy
        epi_transpose(epi_st[last - 1])
        epi_copy(epi_st[last], half=0)
        epi_normalize(epi_st.pop(last - 2))
        epi_copy(epi_st[last], half=1)
        epi_recip(epi_st[last - 1])
        epi_transpose(epi_st[last], half=0)
        epi_normalize(epi_st.pop(last - 1))
        epi_transpose(epi_st[last], half=1)
        epi_recip(epi_st[last])
        epi_normalize(epi_st[last], half=0)
        epi_normalize(epi_st.pop(last), half=1)


_NC_CACHE = None
_TRACE_READY = False


def _enable_tracing():
    """Register the NTFF profile hook that this image's antenv lacks, and
    keep profiling artifacts local instead of uploading to a bucket."""
    global _TRACE_READY
    if _TRACE_READY:
        return
    import sys
    import types

    import antenv
    import concourse.bass_utils as bu
    from trn_agent_boot.trn_boot import _ntff_profile_via_ctypes

    if "antenv.axon_hooks" not in sys.modules:
        mod = types.ModuleType("antenv.axon_hooks")
        mod._hook = None

        def set_axon_ntff_profile_hook(h):
            mod._hook = h

        def get_axon_ntff_profile_hook():
            return mod._hook

        mod.set_axon_ntff_profile_hook = set_axon_ntff_profile_hook
        mod.get_axon_ntff_profile_hook = get_axon_ntff_profile_hook
        sys.modules["antenv.axon_hooks"] = mod
        antenv.axon_hooks = mod

    hooks = sys.modules["antenv.axon_hooks"]
    if hooks.get_axon_ntff_profile_hook() is None:
        hooks.set_axon_ntff_profile_hook(
            _ntff_profile_via_ctypes("/opt/axon/libaxon_pjrt.so")
        )
    bu.upload_artifacts = lambda tmpdir: tmpdir
    _TRACE_READY = True


def _build():
    global _NC_CACHE
    if _NC_CACHE is None:
        nc = bacc.Bacc("TRN2", target_bir_lowering=False, debug=False)
        with tile.TileContext(nc) as tc:
            _attention(tc)
        nc.compile()
        _NC_CACHE = nc
    return _NC_CACHE


def _run(query, key, value, trace=False, tmpdir=None):
    if trace:
        _enable_tracing()
    q = np.ascontiguousarray(np.asarray(query, dtype=np.float32).reshape(B * H, S, D))
    k = np.ascontiguousarray(np.asarray(key, dtype=np.float32).reshape(B * H, S, D))
    v = np.ascontiguousarray(np.asarray(value, dtype=np.float32).reshape(B * H, S, D))
    in_maps = [
        {
            "query": q[c * HPC:(c + 1) * HPC],
            "key": k[c * HPC:(c + 1) * HPC],
            "value": v[c * HPC:(c + 1) * HPC],
        }
        for c in range(N_CORES)
    ]
    nc = _build()
    res = run_bass_kernel_spmd(
        nc, in_maps, core_ids=list(range(N_CORES)), trace=trace, tmpdir=tmpdir
    )
    out = np.stack([res.results[c]["out"] for c in range(N_CORES)])  # [8, HPC, S, D]
    return out.reshape(B, H, S, D), res


def kernel(query, key, value):
    out, _ = _run(query, key, value, trace=bool(int(os.environ.get("BASS_TRACE", "0"))))
    return out
